# revision 25
# baseline (speedup 1.0000x reference)
"""BatchMatchedMSELoss on 8 Trainium2 NeuronCores.

loss = mean(concat(row_min, col_min)) of the (B,B) pairwise-MSE matrix
  mse[i,j] = (||x_i||^2 + ||y_j||^2 - 2 x_i.y_j) / D,  B=8192, D=1024.

Sharding: input rows split across 8 cores (1024 rows each); every core
computes its (1024, 8192) tile of the centered matrix
  cmse[i,j] = D*mse[i,j] - 2048 = (sqx_i-1024) + (sqy_j-1024) - 2 x_i.y_j
via fp8(e4m3) DoubleRow matmuls (K=256/instruction, 2x bf16 TensorE
throughput). Two of the 1024 contraction slots are donated to carry
-0.5*(sqy-1024) as an fp8 hi/lo pair (x-side slots = 1.0), so PSUM already
holds x.y_1022 - 0.5*sqy_c and no per-column vector add is needed later.
The epilogue splits across the remaining engines to hide behind the
matmul stream:
  * Act : evicts PSUM as fp16(-2*psum + sqx_c[m]) (scale/bias fused;
          per-partition bias AP) -> cmse tiles
  * DVE : two fast fp16 tensor_tensor(min) accumulations per tile
          (row accumulator per m, col-min per chunk)
Head: per-core DMA bandwidth (~350 GB/s aggregate over all queues) bounds
when the matmul stream can start, so the head-critical data is minimized:
chunk 0 is only 256 columns and chunk 1 768 (both packed contiguously in
dedicated dram tensors for full-width descriptors), and the X side is
packed m-major and loaded m0-1 first — the stream start is then bounded
by the PE HAM clock-gate warmup (~3.4us of scratch matmuls at 1.2 GHz
cold before 8/8 = 2.4 GHz), not by data. The LAST chunk ships raw
(no DVE work at all): Act -> fp16 tiles -> same-engine Scalar-queue DMAs,
with the row-accumulator partials (final through the second-to-last
chunk) going out on the Sync queue in the same slots, so the 2 MB
row-min side drains during the chunk instead of serializing the tail;
the very last tile computes in 256-column quarters so its Act+DMA
pipeline under the final matmuls. Host folds the raw chunk into both
mins, adds back the exact 2048 offset, and finishes the cross-core /
cross-partition mins, the remaining row reduction, and the mean in fp64.
Measured rel err vs the fp32 reference: 2.6e-4 (tolerance 2e-2).
"""

import numpy as np
import ml_dtypes

import concourse.bass as bass
import concourse.tile as tile
import concourse.mybir as mybir
from concourse.bass_utils import run_bass_kernel_spmd

FP32 = mybir.dt.float32
FP16 = mybir.dt.float16
FP8 = mybir.dt.float8e4
AL = mybir.AluOpType
AF = mybir.ActivationFunctionType

B = 8192          # batch (rows of input and target)
D = 1024          # feature dim (contraction); last 2 slots carry sqy hi/lo
DF = D - 2        # real features used in the fp8 cross product
NCORES = 8
RPC = B // NCORES  # rows per core = 1024
P = 128
MT = RPC // P      # 8 row tiles per core
KG = 4             # DoubleRow k-groups (256 contraction rows each)
CHUNK = 1024       # full column chunk = one PSUM double-bank eviction
HALF = 512         # max moving free dim per matmul / one PSUM bank
C0 = 256           # head chunk 0 (tiny: minimizes head-critical DMA bytes)
C1 = 768           # head chunk 1 (C0 + C1 = one full CHUNK of columns)
# chunk column sizes; chunks 0+1 jointly cover rowacc cols [0, 1024)
CSIZES = [C0, C1] + [CHUNK] * 7
NCH = len(CSIZES)  # 9
COFFS = [sum(CSIZES[:i]) for i in range(NCH)]
QUAR = 256         # last-tile quarter width (shortens the tail Act+DMA)

NP_FP8 = ml_dtypes.float8_e4m3


def _legalize_waits(nc, max_waits=1):
    """walrus codegen in this container rejects instructions carrying more
    than one sync-wait command. Split extra waits onto standalone
    EventSemaphore instructions (same engine, immediately before), which is
    exactly what engine.wait_ge() emits."""
    n = 0
    for f in nc.m.functions:
        for bb in f.blocks:
            insts = bb.instructions
            out = []
            for inst in insts:
                si = inst.sync_info
                if si is not None and si.on_wait and len(si.on_wait) > max_waits:
                    waits = list(si.on_wait)
                    extra, keep = waits[:-max_waits], waits[-max_waits:]
                    for w in extra:
                        n += 1
                        ev = mybir.InstEventSemaphore(
                            name=f"legwait-{n}-{inst.name}", ins=[], outs=[]
                        )
                        ev.engine = inst.engine
                        ev.sync_info = mybir.SyncInfo(on_wait=[w], on_update=[])
                        out.append(ev)
                    inst.sync_info = mybir.SyncInfo(
                        on_wait=keep, on_update=list(si.on_update)
                    )
                out.append(inst)
            bb.instructions = out
    return n


def build_bass(legalize: bool = True) -> bass.Bass:
    nc = bass.Bass()
    # fp8 X operand, m-major DoubleRow layout [P, MT, KG, 2, 128]:
    # element [p, m, kg, s, c] = contraction row kg*256 + s*128 + p of
    # input row m*128 + c. m-major so the first m-tiles land first.
    xtm_d = nc.dram_tensor("xtm", [P, MT, KG, 2, P], FP8, kind="ExternalInput")
    # fp8 Y operands, contraction-major DoubleRow [P, 2, B] per k-group
    yt_d = [
        nc.dram_tensor(f"yt{kg}", [P, 2, B], FP8, kind="ExternalInput")
        for kg in range(KG)
    ]
    # chunks 0/1 packed contiguously (full-width DMA descriptors):
    # [p, kg, s, c] = yt[kg][p, s, c0 + c]
    y0_d = nc.dram_tensor("y0", [P, KG, 2, C0], FP8, kind="ExternalInput")
    y1_d = nc.dram_tensor("y1", [P, KG, 2, C1], FP8, kind="ExternalInput")
    # centered fp32 row sq-norms: sqx[p, m] = |x_{m*128+p}|^2 - 1024
    sqx_d = nc.dram_tensor("sqx", [P, MT], FP32, kind="ExternalInput")
    # per-(p, m) partial row minima over all but the last chunk (host
    # reduces the cols); ships DURING the last chunk, never on the tail
    rowpart_d = nc.dram_tensor("rowpart", [P, MT * CHUNK], FP16, kind="ExternalOutput")
    # column partial mins over this core's 8 m-tiles, all but last chunk
    colmin_d = nc.dram_tensor("colmin", [P, B], FP16, kind="ExternalOutput")
    # ALL of the last chunk ships raw; host folds it into both mins
    mseL_d = nc.dram_tensor("mseL", [P, MT * CHUNK], FP16, kind="ExternalOutput")

    with tile.TileContext(nc) as tc:
        with (
            tc.tile_pool(name="consts", bufs=1) as consts,
            tc.tile_pool(name="ytp", bufs=4) as ytp,
            tc.tile_pool(name="xyp", bufs=4) as xyp,
            tc.tile_pool(name="colp", bufs=3) as colp,
            tc.tile_pool(name="pmm", bufs=4, space=bass.MemorySpace.PSUM) as pmm,
        ):
            sqx = consts.tile([P, MT], FP32)
            rowacc = consts.tile([P, MT * CHUNK], FP16)
            XT = consts.tile([P, MT, KG, 2, P], FP8)
            y0 = consts.tile([P, KG, 2, C0], FP8)
            y1 = consts.tile([P, KG, 2, C1], FP8)

            def xs(m, kg):
                # stationary operand for (m, kg): [128, 2, 128]
                return XT[:, m, kg, :, :]

            # Warmup: a burst of small throwaway matmuls on memset scratch
            # keeps the PE busy from the moment the preamble ends, so the
            # HAM clock gate un-throttles by the time the head operands
            # land. Small moving dim keeps the gating memsets short.
            wx = consts.tile([P, 2, P], FP8)
            wy = consts.tile([P, 2, 256], FP8)
            nc.vector.memset(wx[:], 0)
            nc.vector.memset(wy[:], 0)
            wps = pmm.tile([P, CHUNK], FP32, tag="ps")
            for _ in range(18):
                nc.tensor.matmul(
                    wps[:, 0:256], wx[:], wy[:],
                    start=True, stop=True,
                    perf_mode=mybir.MatmulPerfMode.DoubleRow,
                )
            # Head loads, spread over the three DGE queue engines with the
            # stream-critical bytes first: X m0-1 + all of chunk 0's Y is
            # only 0.5 MB, so the matmul stream start is HAM-bound, not
            # DMA-bound (per-core DMA bandwidth is shared across queues).
            nc.scalar.dma_start(out=XT[:, 0:2], in_=xtm_d[:, 0:2])
            nc.gpsimd.dma_start(out=y0[:], in_=y0_d[:, :, :, :])
            nc.sync.dma_start(out=y1[:], in_=y1_d[:, :, :, :])
            nc.scalar.dma_start(out=XT[:, 2:MT], in_=xtm_d[:, 2:MT])
            nc.gpsimd.dma_start(out=sqx[:], in_=sqx_d[:, :])

            def load_yts(ch):
                j0 = COFFS[ch]
                if ch == 0:
                    return [y0[:, kg, :, :] for kg in range(KG)]
                if ch == 1:
                    return [y1[:, kg, :, :] for kg in range(KG)]
                yts = []
                for kg in range(KG):
                    ytile = ytp.tile(
                        [P, 2, CHUNK], FP8, tag=f"yt{kg}", name=f"yt{kg}"
                    )
                    nc.sync.dma_start(
                        out=ytile[:, :, :],
                        in_=yt_d[kg][:, :, j0 : j0 + CHUNK],
                    )
                    yts.append(ytile[:, :, :])
                return yts

            for ch in range(NCH):
                j0 = COFFS[ch]
                cs = CSIZES[ch]
                yts = load_yts(ch)
                last_ch = ch == NCH - 1
                if last_ch:
                    # Last chunk: no DVE work at all. Every tile ships raw
                    # (Act -> fp16 tile -> same-engine Scalar-queue DMA),
                    # the row-accumulator partials go out on Sync in the
                    # same slots, and the very last tile computes in
                    # 256-col quarters so its Act+DMA pipeline under the
                    # final matmuls. Host folds the raw chunk into both
                    # mins.
                    for m in range(MT):
                        ms = slice(m * CHUNK, (m + 1) * CHUNK)
                        nc.sync.dma_start(
                            out=rowpart_d[:, ms], in_=rowacc[:, ms]
                        )
                        mseL = xyp.tile([P, CHUNK], FP16, tag="mse")
                        if m == MT - 1:
                            for q in range(CHUNK // QUAR):
                                qs = slice(q * QUAR, (q + 1) * QUAR)
                                psq = pmm.tile([P, CHUNK], FP32, tag="ps")
                                for kg in range(KG):
                                    nc.tensor.matmul(
                                        psq[:, 0:QUAR],
                                        xs(m, kg),
                                        yts[kg][:, :, qs],
                                        start=(kg == 0),
                                        stop=(kg == KG - 1),
                                        perf_mode=mybir.MatmulPerfMode.DoubleRow,
                                    )
                                nc.scalar.activation(
                                    mseL[:, qs], psq[:, 0:QUAR], AF.Identity,
                                    bias=sqx[:, m : m + 1], scale=-2.0,
                                )
                                # all but the final quarter ship via GpSimd
                                # so the Scalar engine reaches the last Act
                                # the moment the last matmul lands; the
                                # true tail transfer ships same-engine on
                                # the by-then-empty Scalar queue
                                eng = (nc.scalar if q == CHUNK // QUAR - 1
                                       else nc.gpsimd)
                                eng.dma_start(
                                    out=mseL_d[:, m * CHUNK + q * QUAR :
                                               m * CHUNK + (q + 1) * QUAR],
                                    in_=mseL[:, qs],
                                )
                            continue
                        ps2 = pmm.tile([P, CHUNK], FP32, tag="ps")
                        for h in range(2):
                            hs = slice(h * HALF, (h + 1) * HALF)
                            for kg in range(KG):
                                nc.tensor.matmul(
                                    ps2[:, hs],
                                    xs(m, kg),
                                    yts[kg][:, :, hs],
                                    start=(kg == 0),
                                    stop=(kg == KG - 1),
                                    perf_mode=mybir.MatmulPerfMode.DoubleRow,
                                )
                        nc.scalar.activation(
                            mseL[:], ps2[:, :], AF.Identity,
                            bias=sqx[:, m : m + 1], scale=-2.0,
                        )
                        # alternate issues onto the idle GpSimd queue so
                        # the Scalar engine (Act + DMA issue) stays under
                        # the per-tile matmul budget
                        if m % 2 == 1:
                            nc.gpsimd.dma_start(
                                out=mseL_d[:, ms], in_=mseL[:]
                            )
                        else:
                            nc.scalar.dma_start(
                                out=mseL_d[:, ms], in_=mseL[:]
                            )
                    continue
                colmin = colp.tile([P, CHUNK], FP16, tag="colmin")
                for m in range(MT):
                    # full-CHUNK chunks accumulate into two PSUM banks (two
                    # halves); the small head chunks fit a single group
                    ps2 = pmm.tile([P, CHUNK], FP32, tag="ps")
                    for h0 in range(0, cs, HALF):
                        hsl = slice(h0, min(cs, h0 + HALF))
                        for kg in range(KG):
                            nc.tensor.matmul(
                                ps2[:, hsl],
                                xs(m, kg),
                                yts[kg][:, :, hsl],
                                start=(kg == 0),
                                stop=(kg == KG - 1),
                                perf_mode=mybir.MatmulPerfMode.DoubleRow,
                            )
                    # Act evicts the whole chunk at once as
                    # fp16(-2*psum + sqx_c[m]). Chunks 0/1 write straight
                    # into their disjoint slices of the row accumulator
                    # (no DVE row op needed); for later chunks the first m
                    # writes into the col-min accumulator, saving a DVE
                    # init pass.
                    roff = j0 if ch <= 1 else 0
                    rs = slice(m * CHUNK + roff, m * CHUNK + roff + cs)
                    if ch <= 1:
                        tgt = rowacc[:, rs]
                    elif m == 0:
                        tgt = colmin[:, 0:cs]
                    else:
                        mse = xyp.tile([P, CHUNK], FP16, tag="mse")
                        tgt = mse[:, 0:cs]
                    nc.scalar.activation(
                        tgt, ps2[:, 0:cs], AF.Identity,
                        bias=sqx[:, m : m + 1], scale=-2.0,
                    )
                    # DVE: row accumulator (min across chunks, per m)
                    if ch > 1:
                        nc.vector.tensor_tensor(
                            rowacc[:, rs], rowacc[:, rs], tgt, AL.min
                        )
                    # DVE: col-min accumulator (min across m, per chunk)
                    if ch <= 1 and m == 0:
                        nc.vector.tensor_copy(colmin[:, 0:cs], tgt)
                    elif m > 0:
                        nc.vector.tensor_tensor(
                            colmin[:, 0:cs], colmin[:, 0:cs], tgt, AL.min
                        )

                nc.sync.dma_start(
                    out=colmin_d[:, j0 : j0 + cs], in_=colmin[:, 0:cs]
                )
    if legalize:
        _legalize_waits(nc)
    return nc


_NC_CACHE = None


def _get_nc():
    global _NC_CACHE
    if _NC_CACHE is None:
        _NC_CACHE = build_bass()
    return _NC_CACHE


def _dr_km(t_km: np.ndarray) -> np.ndarray:
    """[D, cols] contraction-major -> [KG, 128, 2, cols] DoubleRow tiles:
    out[kg, p, s, c] = t_km[kg*256 + s*128 + p, c]."""
    d, cols = t_km.shape
    return np.ascontiguousarray(
        t_km.reshape(KG, 2, P, cols).transpose(0, 2, 1, 3)
    )


def _prep_inputs(X, Y):
    """Host-side sharding/layout: fp8 DoubleRow operands with the last two
    contraction slots repurposed to inject -0.5*(sqy-1024) (hi/lo fp8 pair
    against x-side ones), plus centered fp32 sqx rows. Pure layout/dtype
    prep."""
    sqy_c = ((Y.astype(np.float64) ** 2).sum(axis=1) - float(D)).astype(np.float32)
    t = -0.5 * sqy_c
    t_hi = np.clip(t, -224.0, 224.0).astype(NP_FP8)
    t_lo = (t - t_hi.astype(np.float32)).astype(NP_FP8)
    yt_km = np.empty((D, B), dtype=NP_FP8)
    yt_km[:DF] = Y.T[:DF].astype(NP_FP8)
    yt_km[DF] = t_hi
    yt_km[DF + 1] = t_lo
    yq = _dr_km(yt_km)                       # [KG, P, 2, B]
    # packed head chunks: [P, KG, 2, C]
    y0p = np.ascontiguousarray(yq[:, :, :, 0:C0].transpose(1, 0, 2, 3))
    y1p = np.ascontiguousarray(yq[:, :, :, C0 : C0 + C1].transpose(1, 0, 2, 3))

    in_maps = []
    for c in range(NCORES):
        Xs = X[c * RPC : (c + 1) * RPC]
        xt_km = np.empty((D, RPC), dtype=NP_FP8)
        xt_km[:DF] = Xs.T[:DF].astype(NP_FP8)
        xt_km[DF:] = np.float32(1.0)
        xq = _dr_km(xt_km)                   # [KG, P, 2, RPC]
        # m-major pack: [P, MT, KG, 2, 128]
        xtm = np.ascontiguousarray(
            xq.reshape(KG, P, 2, MT, P).transpose(1, 3, 0, 2, 4)
        )
        sqx_c = ((Xs.astype(np.float64) ** 2).sum(axis=1) - float(D)).astype(
            np.float32
        )
        sqx_pm = np.ascontiguousarray(sqx_c.reshape(MT, P).T)
        m = {f"yt{kg}": np.ascontiguousarray(yq[kg]) for kg in range(KG)}
        m.update({"xtm": xtm, "y0": y0p, "y1": y1p, "sqx": sqx_pm})
        in_maps.append(m)
    return in_maps


def kernel(input, target):
    X = np.ascontiguousarray(np.asarray(input, dtype=np.float32))
    Y = np.ascontiguousarray(np.asarray(target, dtype=np.float32))
    assert X.shape == (B, D) and Y.shape == (B, D)

    nc = _get_nc()
    in_maps = _prep_inputs(X, Y)
    try:
        res = run_bass_kernel_spmd(nc, in_maps, core_ids=list(range(NCORES))).results
    except Exception:
        # a prior process can leave a core wedged; one retry clears it
        res = run_bass_kernel_spmd(nc, in_maps, core_ids=list(range(NCORES))).results

    off = np.float64(2.0 * D)
    row_sum = np.float64(0.0)
    col_parts = []
    for r in res:
        # fold the raw last-chunk tiles into both partial-min outputs
        raw = r["mseL"].reshape(P, MT, CHUNK).astype(np.float32)
        rp = r["rowpart"].reshape(P, MT, CHUNK).astype(np.float32).min(axis=2)
        rm = np.minimum(rp, raw.min(axis=2))
        row_sum += (rm.astype(np.float64) + off).sum()
        cm = r["colmin"].astype(np.float32)[:, : B - CHUNK]
        cm7 = raw.min(axis=1)  # [P, CHUNK]: col partial over this core's m
        col_parts.append(np.concatenate([cm.min(axis=0), cm7.min(axis=0)]))
    col_min = np.min(np.stack(col_parts), axis=0).astype(np.float64) + off
    loss = (row_sum + col_min.sum()) / D / (2 * B)
    return np.asarray(loss, dtype=np.float32)


# revision 28
# speedup vs baseline: 1.0319x; 1.0319x over previous
"""BatchMatchedMSELoss on 8 Trainium2 NeuronCores.

loss = mean(concat(row_min, col_min)) of the (B,B) pairwise-MSE matrix
  mse[i,j] = (||x_i||^2 + ||y_j||^2 - 2 x_i.y_j) / D,  B=8192, D=1024.

Sharding: input rows split across 8 cores (1024 rows each); every core
computes its (1024, 8192) tile of the centered matrix
  cmse[i,j] = D*mse[i,j] - 2048 = (sqx_i-1024) + (sqy_j-1024) - 2 x_i.y_j
via fp8(e4m3) DoubleRow matmuls (K=256/instruction, 2x bf16 TensorE
throughput). Two of the 1024 contraction slots are donated to carry
-0.5*(sqy-1024) as an fp8 hi/lo pair (x-side slots = 1.0), so PSUM already
holds x.y_1022 - 0.5*sqy_c and no per-column vector add is needed later.
The epilogue splits across the remaining engines to hide behind the
matmul stream:
  * Act : evicts PSUM as fp16(-2*psum + sqx_c[m]) (scale/bias fused;
          per-partition bias AP) -> cmse tiles
  * DVE : two fast fp16 tensor_tensor(min) accumulations per tile
          (row accumulator per m, col-min per chunk)
Head: per-core DMA bandwidth (~350 GB/s aggregate over all queues) bounds
when the matmul stream can start, so the head-critical data is minimized:
chunk 0 is only 256 columns and chunk 1 768 (both packed contiguously in
dedicated dram tensors for full-width descriptors), and the X side is
packed m-major and loaded m0-1 first — the stream start is then bounded
by the PE HAM clock-gate warmup (~3.4us of scratch matmuls at 1.2 GHz
cold before 8/8 = 2.4 GHz), not by data. The LAST chunk ships raw
(no DVE work at all): Act -> fp16 tiles -> same-engine Scalar-queue DMAs,
with the row-accumulator partials (final through the second-to-last
chunk) going out on the Sync queue in the same slots, so the 2 MB
row-min side drains during the chunk instead of serializing the tail;
the very last tile computes in 256-column quarters so its Act+DMA
pipeline under the final matmuls. Host folds the raw chunk into both
mins, adds back the exact 2048 offset, and finishes the cross-core /
cross-partition mins, the remaining row reduction, and the mean in fp64.
Measured rel err vs the fp32 reference: 2.6e-4 (tolerance 2e-2).
"""

import numpy as np
import ml_dtypes

import concourse.bass as bass
import concourse.tile as tile
import concourse.mybir as mybir
from concourse.bass_utils import run_bass_kernel_spmd

FP32 = mybir.dt.float32
FP16 = mybir.dt.float16
FP8 = mybir.dt.float8e4
AL = mybir.AluOpType
AF = mybir.ActivationFunctionType

B = 8192          # batch (rows of input and target)
D = 1024          # feature dim (contraction); last 2 slots carry sqy hi/lo
DF = D - 2        # real features used in the fp8 cross product
NCORES = 8
RPC = B // NCORES  # rows per core = 1024
P = 128
MT = RPC // P      # 8 row tiles per core
KG = 4             # DoubleRow k-groups (256 contraction rows each)
CHUNK = 1024       # full column chunk = one PSUM double-bank eviction
HALF = 512         # max moving free dim per matmul / one PSUM bank
C0 = 256           # head chunk 0 (tiny: minimizes head-critical DMA bytes)
C1 = 768           # head chunk 1 (C0 + C1 = one full CHUNK of columns)
# chunk column sizes; chunks 0+1 jointly cover rowacc cols [0, 1024)
CSIZES = [C0, C1] + [CHUNK] * 7
NCH = len(CSIZES)  # 9
COFFS = [sum(CSIZES[:i]) for i in range(NCH)]
QUAR = 256         # last-tile quarter width (shortens the tail Act+DMA)

NP_FP8 = ml_dtypes.float8_e4m3


def _legalize_waits(nc, max_waits=1):
    """walrus codegen in this container rejects instructions carrying more
    than one sync-wait command. Split extra waits onto standalone
    EventSemaphore instructions (same engine, immediately before), which is
    exactly what engine.wait_ge() emits."""
    n = 0
    for f in nc.m.functions:
        for bb in f.blocks:
            insts = bb.instructions
            out = []
            for inst in insts:
                si = inst.sync_info
                if si is not None and si.on_wait and len(si.on_wait) > max_waits:
                    waits = list(si.on_wait)
                    extra, keep = waits[:-max_waits], waits[-max_waits:]
                    for w in extra:
                        n += 1
                        ev = mybir.InstEventSemaphore(
                            name=f"legwait-{n}-{inst.name}", ins=[], outs=[]
                        )
                        ev.engine = inst.engine
                        ev.sync_info = mybir.SyncInfo(on_wait=[w], on_update=[])
                        out.append(ev)
                    inst.sync_info = mybir.SyncInfo(
                        on_wait=keep, on_update=list(si.on_update)
                    )
                out.append(inst)
            bb.instructions = out
    return n


def build_bass(legalize: bool = True) -> bass.Bass:
    nc = bass.Bass()
    # fp8 X operand, m-major DoubleRow layout [P, MT, KG, 2, 128]:
    # element [p, m, kg, s, c] = contraction row kg*256 + s*128 + p of
    # input row m*128 + c. m-major so the first m-tiles land first.
    xtm_d = nc.dram_tensor("xtm", [P, MT, KG, 2, P], FP8, kind="ExternalInput")
    # fp8 Y operands, contraction-major DoubleRow [P, 2, B] per k-group
    yt_d = [
        nc.dram_tensor(f"yt{kg}", [P, 2, B], FP8, kind="ExternalInput")
        for kg in range(KG)
    ]
    # chunks 0/1 packed contiguously (full-width DMA descriptors):
    # [p, kg, s, c] = yt[kg][p, s, c0 + c]
    y0_d = nc.dram_tensor("y0", [P, KG, 2, C0], FP8, kind="ExternalInput")
    y1_d = nc.dram_tensor("y1", [P, KG, 2, C1], FP8, kind="ExternalInput")
    # centered fp32 row sq-norms: sqx[p, m] = |x_{m*128+p}|^2 - 1024
    sqx_d = nc.dram_tensor("sqx", [P, MT], FP32, kind="ExternalInput")
    # per-(p, m) partial row minima over all but the last chunk (host
    # reduces the cols); ships DURING the last chunk, never on the tail
    rowpart_d = nc.dram_tensor("rowpart", [P, MT * CHUNK], FP16, kind="ExternalOutput")
    # column partial mins over this core's 8 m-tiles, all but last chunk
    colmin_d = nc.dram_tensor("colmin", [P, B], FP16, kind="ExternalOutput")
    # ALL of the last chunk ships raw; host folds it into both mins
    mseL_d = nc.dram_tensor("mseL", [P, MT * CHUNK], FP16, kind="ExternalOutput")

    with tile.TileContext(nc) as tc:
        with (
            tc.tile_pool(name="consts", bufs=1) as consts,
            tc.tile_pool(name="ytp", bufs=4) as ytp,
            tc.tile_pool(name="xyp", bufs=4) as xyp,
            tc.tile_pool(name="colp", bufs=3) as colp,
            tc.tile_pool(name="pmm", bufs=4, space=bass.MemorySpace.PSUM) as pmm,
        ):
            sqx = consts.tile([P, MT], FP32)
            rowacc = consts.tile([P, MT * CHUNK], FP16)
            XT = consts.tile([P, MT, KG, 2, P], FP8)
            y0 = consts.tile([P, KG, 2, C0], FP8)
            y1 = consts.tile([P, KG, 2, C1], FP8)

            def xs(m, kg):
                # stationary operand for (m, kg): [128, 2, 128]
                return XT[:, m, kg, :, :]

            # Warmup: a burst of small throwaway matmuls on memset scratch
            # keeps the PE busy from the moment the preamble ends, so the
            # HAM clock gate un-throttles by the time the head operands
            # land. Small moving dim keeps the gating memsets short.
            wx = consts.tile([P, 2, P], FP8)
            wy = consts.tile([P, 2, 256], FP8)
            nc.vector.memset(wx[:], 0)
            nc.vector.memset(wy[:], 0)
            wps = pmm.tile([P, CHUNK], FP32, tag="ps")
            for _ in range(12):
                nc.tensor.matmul(
                    wps[:, 0:256], wx[:], wy[:],
                    start=True, stop=True,
                    perf_mode=mybir.MatmulPerfMode.DoubleRow,
                )
            # Head loads on the two HARDWARE DGE queues only (GpSimd DMAs
            # take a software descriptor path with multi-us trigger latency
            # — never put stream-critical loads there), stream-critical
            # bytes first: X m0-1 + all of chunk 0's Y is only 0.5 MB, so
            # the matmul stream start is HAM-bound, not DMA-bound (per-core
            # DMA bandwidth is shared across queues). X ships in three
            # pieces so completion semaphores fire as the early m-tiles
            # land rather than only after the full megabyte.
            nc.scalar.dma_start(out=XT[:, 0:2], in_=xtm_d[:, 0:2])
            nc.sync.dma_start(out=y0[:], in_=y0_d[:, :, :, :])
            nc.scalar.dma_start(out=XT[:, 2:5], in_=xtm_d[:, 2:5])
            nc.sync.dma_start(out=y1[:], in_=y1_d[:, :, :, :])
            nc.scalar.dma_start(out=XT[:, 5:MT], in_=xtm_d[:, 5:MT])
            nc.sync.dma_start(out=sqx[:], in_=sqx_d[:, :])

            def load_yts(ch):
                j0 = COFFS[ch]
                if ch == 0:
                    return [y0[:, kg, :, :] for kg in range(KG)]
                if ch == 1:
                    return [y1[:, kg, :, :] for kg in range(KG)]
                yts = []
                for kg in range(KG):
                    ytile = ytp.tile(
                        [P, 2, CHUNK], FP8, tag=f"yt{kg}", name=f"yt{kg}"
                    )
                    nc.sync.dma_start(
                        out=ytile[:, :, :],
                        in_=yt_d[kg][:, :, j0 : j0 + CHUNK],
                    )
                    yts.append(ytile[:, :, :])
                return yts

            for ch in range(NCH):
                j0 = COFFS[ch]
                cs = CSIZES[ch]
                yts = load_yts(ch)
                last_ch = ch == NCH - 1
                if last_ch:
                    # Last chunk: no DVE work at all. Every tile ships raw
                    # (Act -> fp16 tile -> same-engine Scalar-queue DMA),
                    # the row-accumulator partials go out on Sync in the
                    # same slots, and the very last tile computes in
                    # 256-col quarters so its Act+DMA pipeline under the
                    # final matmuls. Host folds the raw chunk into both
                    # mins.
                    for m in range(MT):
                        ms = slice(m * CHUNK, (m + 1) * CHUNK)
                        nc.sync.dma_start(
                            out=rowpart_d[:, ms], in_=rowacc[:, ms]
                        )
                        mseL = xyp.tile([P, CHUNK], FP16, tag="mse")
                        if m == MT - 1:
                            for q in range(CHUNK // QUAR):
                                qs = slice(q * QUAR, (q + 1) * QUAR)
                                psq = pmm.tile([P, CHUNK], FP32, tag="ps")
                                for kg in range(KG):
                                    nc.tensor.matmul(
                                        psq[:, 0:QUAR],
                                        xs(m, kg),
                                        yts[kg][:, :, qs],
                                        start=(kg == 0),
                                        stop=(kg == KG - 1),
                                        perf_mode=mybir.MatmulPerfMode.DoubleRow,
                                    )
                                nc.scalar.activation(
                                    mseL[:, qs], psq[:, 0:QUAR], AF.Identity,
                                    bias=sqx[:, m : m + 1], scale=-2.0,
                                )
                                # all but the final quarter ship via Sync
                                # so the Scalar engine reaches the last Act
                                # the moment the last matmul lands; the
                                # true tail transfer ships same-engine on
                                # the by-then-empty Scalar queue
                                eng = (nc.scalar if q == CHUNK // QUAR - 1
                                       else nc.sync)
                                eng.dma_start(
                                    out=mseL_d[:, m * CHUNK + q * QUAR :
                                               m * CHUNK + (q + 1) * QUAR],
                                    in_=mseL[:, qs],
                                )
                            continue
                        ps2 = pmm.tile([P, CHUNK], FP32, tag="ps")
                        for h in range(2):
                            hs = slice(h * HALF, (h + 1) * HALF)
                            for kg in range(KG):
                                nc.tensor.matmul(
                                    ps2[:, hs],
                                    xs(m, kg),
                                    yts[kg][:, :, hs],
                                    start=(kg == 0),
                                    stop=(kg == KG - 1),
                                    perf_mode=mybir.MatmulPerfMode.DoubleRow,
                                )
                        nc.scalar.activation(
                            mseL[:], ps2[:, :], AF.Identity,
                            bias=sqx[:, m : m + 1], scale=-2.0,
                        )
                        # alternate issues onto the Sync queue so the
                        # Scalar engine (Act + DMA issue) stays under the
                        # per-tile matmul budget (GpSimd's software-DGE
                        # latency would push completions past the final
                        # barrier)
                        if m % 2 == 1:
                            nc.sync.dma_start(
                                out=mseL_d[:, ms], in_=mseL[:]
                            )
                        else:
                            nc.scalar.dma_start(
                                out=mseL_d[:, ms], in_=mseL[:]
                            )
                    continue
                colmin = colp.tile([P, CHUNK], FP16, tag="colmin")
                for m in range(MT):
                    # full-CHUNK chunks accumulate into two PSUM banks (two
                    # halves); the small head chunks fit a single group
                    ps2 = pmm.tile([P, CHUNK], FP32, tag="ps")
                    for h0 in range(0, cs, HALF):
                        hsl = slice(h0, min(cs, h0 + HALF))
                        for kg in range(KG):
                            nc.tensor.matmul(
                                ps2[:, hsl],
                                xs(m, kg),
                                yts[kg][:, :, hsl],
                                start=(kg == 0),
                                stop=(kg == KG - 1),
                                perf_mode=mybir.MatmulPerfMode.DoubleRow,
                            )
                    # Act evicts the whole chunk at once as
                    # fp16(-2*psum + sqx_c[m]). Chunks 0/1 write straight
                    # into their disjoint slices of the row accumulator
                    # (no DVE row op needed); for later chunks the first m
                    # writes into the col-min accumulator, saving a DVE
                    # init pass.
                    roff = j0 if ch <= 1 else 0
                    rs = slice(m * CHUNK + roff, m * CHUNK + roff + cs)
                    if ch <= 1:
                        tgt = rowacc[:, rs]
                    elif m == 0:
                        tgt = colmin[:, 0:cs]
                    else:
                        mse = xyp.tile([P, CHUNK], FP16, tag="mse")
                        tgt = mse[:, 0:cs]
                    nc.scalar.activation(
                        tgt, ps2[:, 0:cs], AF.Identity,
                        bias=sqx[:, m : m + 1], scale=-2.0,
                    )
                    # DVE: row accumulator (min across chunks, per m)
                    if ch > 1:
                        nc.vector.tensor_tensor(
                            rowacc[:, rs], rowacc[:, rs], tgt, AL.min
                        )
                    # DVE: col-min accumulator (min across m, per chunk)
                    if ch <= 1 and m == 0:
                        nc.vector.tensor_copy(colmin[:, 0:cs], tgt)
                    elif m > 0:
                        nc.vector.tensor_tensor(
                            colmin[:, 0:cs], colmin[:, 0:cs], tgt, AL.min
                        )

                nc.sync.dma_start(
                    out=colmin_d[:, j0 : j0 + cs], in_=colmin[:, 0:cs]
                )
    if legalize:
        _legalize_waits(nc)
    return nc


_NC_CACHE = None


def _get_nc():
    global _NC_CACHE
    if _NC_CACHE is None:
        _NC_CACHE = build_bass()
    return _NC_CACHE


def _dr_km(t_km: np.ndarray) -> np.ndarray:
    """[D, cols] contraction-major -> [KG, 128, 2, cols] DoubleRow tiles:
    out[kg, p, s, c] = t_km[kg*256 + s*128 + p, c]."""
    d, cols = t_km.shape
    return np.ascontiguousarray(
        t_km.reshape(KG, 2, P, cols).transpose(0, 2, 1, 3)
    )


def _prep_inputs(X, Y):
    """Host-side sharding/layout: fp8 DoubleRow operands with the last two
    contraction slots repurposed to inject -0.5*(sqy-1024) (hi/lo fp8 pair
    against x-side ones), plus centered fp32 sqx rows. Pure layout/dtype
    prep."""
    sqy_c = ((Y.astype(np.float64) ** 2).sum(axis=1) - float(D)).astype(np.float32)
    t = -0.5 * sqy_c
    t_hi = np.clip(t, -224.0, 224.0).astype(NP_FP8)
    t_lo = (t - t_hi.astype(np.float32)).astype(NP_FP8)
    yt_km = np.empty((D, B), dtype=NP_FP8)
    yt_km[:DF] = Y.T[:DF].astype(NP_FP8)
    yt_km[DF] = t_hi
    yt_km[DF + 1] = t_lo
    yq = _dr_km(yt_km)                       # [KG, P, 2, B]
    # packed head chunks: [P, KG, 2, C]
    y0p = np.ascontiguousarray(yq[:, :, :, 0:C0].transpose(1, 0, 2, 3))
    y1p = np.ascontiguousarray(yq[:, :, :, C0 : C0 + C1].transpose(1, 0, 2, 3))

    in_maps = []
    for c in range(NCORES):
        Xs = X[c * RPC : (c + 1) * RPC]
        xt_km = np.empty((D, RPC), dtype=NP_FP8)
        xt_km[:DF] = Xs.T[:DF].astype(NP_FP8)
        xt_km[DF:] = np.float32(1.0)
        xq = _dr_km(xt_km)                   # [KG, P, 2, RPC]
        # m-major pack: [P, MT, KG, 2, 128]
        xtm = np.ascontiguousarray(
            xq.reshape(KG, P, 2, MT, P).transpose(1, 3, 0, 2, 4)
        )
        sqx_c = ((Xs.astype(np.float64) ** 2).sum(axis=1) - float(D)).astype(
            np.float32
        )
        sqx_pm = np.ascontiguousarray(sqx_c.reshape(MT, P).T)
        m = {f"yt{kg}": np.ascontiguousarray(yq[kg]) for kg in range(KG)}
        m.update({"xtm": xtm, "y0": y0p, "y1": y1p, "sqx": sqx_pm})
        in_maps.append(m)
    return in_maps


def kernel(input, target):
    X = np.ascontiguousarray(np.asarray(input, dtype=np.float32))
    Y = np.ascontiguousarray(np.asarray(target, dtype=np.float32))
    assert X.shape == (B, D) and Y.shape == (B, D)

    nc = _get_nc()
    in_maps = _prep_inputs(X, Y)
    try:
        res = run_bass_kernel_spmd(nc, in_maps, core_ids=list(range(NCORES))).results
    except Exception:
        # a prior process can leave a core wedged; one retry clears it
        res = run_bass_kernel_spmd(nc, in_maps, core_ids=list(range(NCORES))).results

    off = np.float64(2.0 * D)
    row_sum = np.float64(0.0)
    col_parts = []
    for r in res:
        # fold the raw last-chunk tiles into both partial-min outputs
        raw = r["mseL"].reshape(P, MT, CHUNK).astype(np.float32)
        rp = r["rowpart"].reshape(P, MT, CHUNK).astype(np.float32).min(axis=2)
        rm = np.minimum(rp, raw.min(axis=2))
        row_sum += (rm.astype(np.float64) + off).sum()
        cm = r["colmin"].astype(np.float32)[:, : B - CHUNK]
        cm7 = raw.min(axis=1)  # [P, CHUNK]: col partial over this core's m
        col_parts.append(np.concatenate([cm.min(axis=0), cm7.min(axis=0)]))
    col_min = np.min(np.stack(col_parts), axis=0).astype(np.float64) + off
    loss = (row_sum + col_min.sum()) / D / (2 * B)
    return np.asarray(loss, dtype=np.float32)


# revision 31
# speedup vs baseline: 1.0465x; 1.0142x over previous
"""BatchMatchedMSELoss on 8 Trainium2 NeuronCores.

loss = mean(concat(row_min, col_min)) of the (B,B) pairwise-MSE matrix
  mse[i,j] = (||x_i||^2 + ||y_j||^2 - 2 x_i.y_j) / D,  B=8192, D=1024.

Sharding: input rows split across 8 cores (1024 rows each); every core
computes its (1024, 8192) tile of the centered matrix
  cmse[i,j] = D*mse[i,j] - 2048 = (sqx_i-1024) + (sqy_j-1024) - 2 x_i.y_j
via fp8(e4m3) DoubleRow matmuls (K=256/instruction, 2x bf16 TensorE
throughput). Two of the 1024 contraction slots are donated to carry
-0.5*(sqy-1024) as an fp8 hi/lo pair (x-side slots = 1.0), so PSUM already
holds x.y_1022 - 0.5*sqy_c and no per-column vector add is needed later.
The epilogue splits across the remaining engines to hide behind the
matmul stream:
  * Act : evicts PSUM as fp16(-2*psum + sqx_c[m]) (scale/bias fused;
          per-partition bias AP) -> cmse tiles
  * DVE : two fast fp16 tensor_tensor(min) accumulations per tile
          (row accumulator per m, col-min per chunk)
Head: per-core DMA bandwidth (~350 GB/s aggregate over all queues) bounds
when the matmul stream can start, so the head-critical data is minimized:
chunk 0 is only 256 columns and chunk 1 768 (both packed contiguously in
dedicated dram tensors for full-width descriptors), and the X side is
packed m-major and loaded m0-1 first — the stream start is then bounded
by the PE HAM clock-gate warmup (~3.4us of scratch matmuls at 1.2 GHz
cold before 8/8 = 2.4 GHz), not by data. The LAST chunk ships raw
(no DVE work at all): Act -> fp16 tiles -> same-engine Scalar-queue DMAs,
with the row-accumulator partials (final through the second-to-last
chunk) going out on the Sync queue in the same slots, so the 2 MB
row-min side drains during the chunk instead of serializing the tail;
the very last tile computes in 256-column quarters so its Act+DMA
pipeline under the final matmuls. Host folds the raw chunk into both
mins, adds back the exact 2048 offset, and finishes the cross-core /
cross-partition mins, the remaining row reduction, and the mean in fp64.
Measured rel err vs the fp32 reference: 2.6e-4 (tolerance 2e-2).
"""

import numpy as np
import ml_dtypes

import concourse.bass as bass
import concourse.tile as tile
import concourse.mybir as mybir
from concourse.bass_utils import run_bass_kernel_spmd

FP32 = mybir.dt.float32
FP16 = mybir.dt.float16
FP8 = mybir.dt.float8e4
AL = mybir.AluOpType
AF = mybir.ActivationFunctionType

B = 8192          # batch (rows of input and target)
D = 1024          # feature dim (contraction); last 2 slots carry sqy hi/lo
DF = D - 2        # real features used in the fp8 cross product
NCORES = 8
RPC = B // NCORES  # rows per core = 1024
P = 128
MT = RPC // P      # 8 row tiles per core
KG = 4             # DoubleRow k-groups (256 contraction rows each)
CHUNK = 1024       # full column chunk = one PSUM double-bank eviction
HALF = 512         # max moving free dim per matmul / one PSUM bank
C0 = 256           # head chunk 0 (tiny: minimizes head-critical DMA bytes)
C1 = 768           # head chunk 1 (C0 + C1 = one full CHUNK of columns)
# chunk column sizes; chunks 0+1 jointly cover rowacc cols [0, 1024)
CSIZES = [C0, C1] + [CHUNK] * 7
NCH = len(CSIZES)  # 9
COFFS = [sum(CSIZES[:i]) for i in range(NCH)]
QUAR = 256         # last-tile quarter width (shortens the tail Act+DMA)

NP_FP8 = ml_dtypes.float8_e4m3


def _legalize_waits(nc, max_waits=1):
    """walrus codegen in this container rejects instructions carrying more
    than one sync-wait command. Split extra waits onto standalone
    EventSemaphore instructions (same engine, immediately before), which is
    exactly what engine.wait_ge() emits."""
    n = 0
    for f in nc.m.functions:
        for bb in f.blocks:
            insts = bb.instructions
            out = []
            for inst in insts:
                si = inst.sync_info
                if si is not None and si.on_wait and len(si.on_wait) > max_waits:
                    waits = list(si.on_wait)
                    extra, keep = waits[:-max_waits], waits[-max_waits:]
                    for w in extra:
                        n += 1
                        ev = mybir.InstEventSemaphore(
                            name=f"legwait-{n}-{inst.name}", ins=[], outs=[]
                        )
                        ev.engine = inst.engine
                        ev.sync_info = mybir.SyncInfo(on_wait=[w], on_update=[])
                        out.append(ev)
                    inst.sync_info = mybir.SyncInfo(
                        on_wait=keep, on_update=list(si.on_update)
                    )
                out.append(inst)
            bb.instructions = out
    return n


def build_bass(legalize: bool = True) -> bass.Bass:
    nc = bass.Bass()
    # fp8 X operand, m-major DoubleRow layout [P, MT, KG, 2, 128]:
    # element [p, m, kg, s, c] = contraction row kg*256 + s*128 + p of
    # input row m*128 + c. m-major so the first m-tiles land first.
    xtm_d = nc.dram_tensor("xtm", [P, MT, KG, 2, P], FP8, kind="ExternalInput")
    # fp8 Y operands, contraction-major DoubleRow [P, 2, B] per k-group
    yt_d = [
        nc.dram_tensor(f"yt{kg}", [P, 2, B], FP8, kind="ExternalInput")
        for kg in range(KG)
    ]
    # chunks 0/1 packed contiguously (full-width DMA descriptors):
    # [p, kg, s, c] = yt[kg][p, s, c0 + c]
    y0_d = nc.dram_tensor("y0", [P, KG, 2, C0], FP8, kind="ExternalInput")
    y1_d = nc.dram_tensor("y1", [P, KG, 2, C1], FP8, kind="ExternalInput")
    # centered fp32 row sq-norms: sqx[p, m] = |x_{m*128+p}|^2 - 1024
    sqx_d = nc.dram_tensor("sqx", [P, MT], FP32, kind="ExternalInput")
    # per-(p, m) partial row minima over all but the last chunk (host
    # reduces the cols); ships DURING the last chunk, never on the tail
    rowpart_d = nc.dram_tensor("rowpart", [P, MT * CHUNK], FP16, kind="ExternalOutput")
    # column partial mins over this core's 8 m-tiles, all but last chunk
    colmin_d = nc.dram_tensor("colmin", [P, B], FP16, kind="ExternalOutput")
    # ALL of the last chunk ships raw; host folds it into both mins
    mseL_d = nc.dram_tensor("mseL", [P, MT * CHUNK], FP16, kind="ExternalOutput")

    with tile.TileContext(nc) as tc:
        with (
            tc.tile_pool(name="consts", bufs=1) as consts,
            tc.tile_pool(name="ytp", bufs=4) as ytp,
            tc.tile_pool(name="xyp", bufs=4) as xyp,
            tc.tile_pool(name="colp", bufs=3) as colp,
            tc.tile_pool(name="pmm", bufs=4, space=bass.MemorySpace.PSUM) as pmm,
        ):
            sqx = consts.tile([P, MT], FP32)
            rowacc = consts.tile([P, MT * CHUNK], FP16)
            XT = consts.tile([P, MT, KG, 2, P], FP8)
            y0 = consts.tile([P, KG, 2, C0], FP8)
            y1 = consts.tile([P, KG, 2, C1], FP8)

            def xs(m, kg):
                # stationary operand for (m, kg): [128, 2, 128]
                return XT[:, m, kg, :, :]

            # Warmup: a burst of small throwaway matmuls on memset scratch
            # keeps the PE busy from the moment the preamble ends, so the
            # HAM clock gate un-throttles by the time the head operands
            # land. Small moving dim keeps the gating memsets short.
            wx = consts.tile([P, 2, P], FP8)
            wy = consts.tile([P, 2, 256], FP8)
            nc.vector.memset(wx[:], 0)
            nc.vector.memset(wy[:], 0)
            wps = pmm.tile([P, CHUNK], FP32, tag="ps")
            for _ in range(24):
                nc.tensor.matmul(
                    wps[:, 0:256], wx[:], wy[:],
                    start=True, stop=True,
                    perf_mode=mybir.MatmulPerfMode.DoubleRow,
                )
            # Head loads on the two HARDWARE DGE queues only (GpSimd DMAs
            # take a software descriptor path with multi-us trigger latency
            # — never put stream-critical loads there), stream-critical
            # bytes first: X m0-1 + all of chunk 0's Y is only 0.5 MB, so
            # the matmul stream start is HAM-bound, not DMA-bound (per-core
            # DMA bandwidth is shared across queues). X ships in three
            # pieces so completion semaphores fire as the early m-tiles
            # land rather than only after the full megabyte.
            nc.scalar.dma_start(out=XT[:, 0:2], in_=xtm_d[:, 0:2])
            nc.sync.dma_start(out=y0[:], in_=y0_d[:, :, :, :])
            nc.scalar.dma_start(out=XT[:, 2:5], in_=xtm_d[:, 2:5])
            nc.sync.dma_start(out=y1[:], in_=y1_d[:, :, :, :])
            nc.scalar.dma_start(out=XT[:, 5:MT], in_=xtm_d[:, 5:MT])
            nc.sync.dma_start(out=sqx[:], in_=sqx_d[:, :])

            def load_yts(ch):
                j0 = COFFS[ch]
                if ch == 0:
                    return [y0[:, kg, :, :] for kg in range(KG)]
                if ch == 1:
                    return [y1[:, kg, :, :] for kg in range(KG)]
                yts = []
                for kg in range(KG):
                    ytile = ytp.tile(
                        [P, 2, CHUNK], FP8, tag=f"yt{kg}", name=f"yt{kg}"
                    )
                    nc.sync.dma_start(
                        out=ytile[:, :, :],
                        in_=yt_d[kg][:, :, j0 : j0 + CHUNK],
                    )
                    yts.append(ytile[:, :, :])
                return yts

            for ch in range(NCH):
                j0 = COFFS[ch]
                cs = CSIZES[ch]
                yts = load_yts(ch)
                last_ch = ch == NCH - 1
                if last_ch:
                    # Last chunk: no DVE work at all. Every tile ships raw
                    # (Act -> fp16 tile -> same-engine Scalar-queue DMA),
                    # the row-accumulator partials go out on Sync in the
                    # same slots, and the very last tile computes in
                    # 256-col quarters so its Act+DMA pipeline under the
                    # final matmuls. Host folds the raw chunk into both
                    # mins.
                    for m in range(MT):
                        ms = slice(m * CHUNK, (m + 1) * CHUNK)
                        nc.sync.dma_start(
                            out=rowpart_d[:, ms], in_=rowacc[:, ms]
                        )
                        mseL = xyp.tile([P, CHUNK], FP16, tag="mse")
                        if m == MT - 1:
                            for h in range(2):
                                hs = slice(h * HALF, (h + 1) * HALF)
                                # separate PSUM tiles per half: no false
                                # h1-vs-h0 wait through a shared tile
                                psh = pmm.tile([P, CHUNK], FP32, tag="ps")
                                for kg in range(KG):
                                    nc.tensor.matmul(
                                        psh[:, 0:HALF],
                                        xs(m, kg),
                                        yts[kg][:, :, hs],
                                        start=(kg == 0),
                                        stop=(kg == KG - 1),
                                        perf_mode=mybir.MatmulPerfMode.DoubleRow,
                                    )
                                nc.scalar.activation(
                                    mseL[:, hs], psh[:, 0:HALF], AF.Identity,
                                    bias=sqx[:, m : m + 1], scale=-2.0,
                                )
                                # h0 ships via Sync so the Scalar engine
                                # reaches h1's Act the moment the last
                                # matmul lands; the true tail transfer
                                # ships same-engine on the by-then-empty
                                # Scalar queue
                                eng = nc.scalar if h == 1 else nc.sync
                                eng.dma_start(
                                    out=mseL_d[:, m * CHUNK + h * HALF :
                                               m * CHUNK + (h + 1) * HALF],
                                    in_=mseL[:, hs],
                                )
                            continue
                        ps2 = pmm.tile([P, CHUNK], FP32, tag="ps")
                        for h in range(2):
                            hs = slice(h * HALF, (h + 1) * HALF)
                            for kg in range(KG):
                                nc.tensor.matmul(
                                    ps2[:, hs],
                                    xs(m, kg),
                                    yts[kg][:, :, hs],
                                    start=(kg == 0),
                                    stop=(kg == KG - 1),
                                    perf_mode=mybir.MatmulPerfMode.DoubleRow,
                                )
                        nc.scalar.activation(
                            mseL[:], ps2[:, :], AF.Identity,
                            bias=sqx[:, m : m + 1], scale=-2.0,
                        )
                        # alternate issues onto the Sync queue so the
                        # Scalar engine (Act + DMA issue) stays under the
                        # per-tile matmul budget (GpSimd's software-DGE
                        # latency would push completions past the final
                        # barrier); m6 also goes to Sync so the Scalar
                        # engine is free when the last tile's halves land
                        if m % 2 == 1 or m == MT - 2:
                            nc.sync.dma_start(
                                out=mseL_d[:, ms], in_=mseL[:]
                            )
                        else:
                            nc.scalar.dma_start(
                                out=mseL_d[:, ms], in_=mseL[:]
                            )
                    continue
                colmin = colp.tile([P, CHUNK], FP16, tag="colmin")
                for m in range(MT):
                    # full-CHUNK chunks accumulate into two PSUM banks (two
                    # halves); the small head chunks fit a single group
                    ps2 = pmm.tile([P, CHUNK], FP32, tag="ps")
                    for h0 in range(0, cs, HALF):
                        hsl = slice(h0, min(cs, h0 + HALF))
                        for kg in range(KG):
                            nc.tensor.matmul(
                                ps2[:, hsl],
                                xs(m, kg),
                                yts[kg][:, :, hsl],
                                start=(kg == 0),
                                stop=(kg == KG - 1),
                                perf_mode=mybir.MatmulPerfMode.DoubleRow,
                            )
                    # Act evicts the whole chunk at once as
                    # fp16(-2*psum + sqx_c[m]). Chunks 0/1 write straight
                    # into their disjoint slices of the row accumulator
                    # (no DVE row op needed); for later chunks the first m
                    # writes into the col-min accumulator, saving a DVE
                    # init pass.
                    roff = j0 if ch <= 1 else 0
                    rs = slice(m * CHUNK + roff, m * CHUNK + roff + cs)
                    if ch <= 1:
                        tgt = rowacc[:, rs]
                    elif m == 0:
                        tgt = colmin[:, 0:cs]
                    else:
                        mse = xyp.tile([P, CHUNK], FP16, tag="mse")
                        tgt = mse[:, 0:cs]
                    nc.scalar.activation(
                        tgt, ps2[:, 0:cs], AF.Identity,
                        bias=sqx[:, m : m + 1], scale=-2.0,
                    )
                    # DVE: row accumulator (min across chunks, per m)
                    if ch > 1:
                        nc.vector.tensor_tensor(
                            rowacc[:, rs], rowacc[:, rs], tgt, AL.min
                        )
                    # DVE: col-min accumulator (min across m, per chunk)
                    if ch <= 1 and m == 0:
                        nc.vector.tensor_copy(colmin[:, 0:cs], tgt)
                    elif m > 0:
                        nc.vector.tensor_tensor(
                            colmin[:, 0:cs], colmin[:, 0:cs], tgt, AL.min
                        )

                nc.sync.dma_start(
                    out=colmin_d[:, j0 : j0 + cs], in_=colmin[:, 0:cs]
                )
    if legalize:
        _legalize_waits(nc)
    return nc


_NC_CACHE = None


def _get_nc():
    global _NC_CACHE
    if _NC_CACHE is None:
        _NC_CACHE = build_bass()
    return _NC_CACHE


def _dr_km(t_km: np.ndarray) -> np.ndarray:
    """[D, cols] contraction-major -> [KG, 128, 2, cols] DoubleRow tiles:
    out[kg, p, s, c] = t_km[kg*256 + s*128 + p, c]."""
    d, cols = t_km.shape
    return np.ascontiguousarray(
        t_km.reshape(KG, 2, P, cols).transpose(0, 2, 1, 3)
    )


def _prep_inputs(X, Y):
    """Host-side sharding/layout: fp8 DoubleRow operands with the last two
    contraction slots repurposed to inject -0.5*(sqy-1024) (hi/lo fp8 pair
    against x-side ones), plus centered fp32 sqx rows. Pure layout/dtype
    prep."""
    sqy_c = ((Y.astype(np.float64) ** 2).sum(axis=1) - float(D)).astype(np.float32)
    t = -0.5 * sqy_c
    t_hi = np.clip(t, -224.0, 224.0).astype(NP_FP8)
    t_lo = (t - t_hi.astype(np.float32)).astype(NP_FP8)
    yt_km = np.empty((D, B), dtype=NP_FP8)
    yt_km[:DF] = Y.T[:DF].astype(NP_FP8)
    yt_km[DF] = t_hi
    yt_km[DF + 1] = t_lo
    yq = _dr_km(yt_km)                       # [KG, P, 2, B]
    # packed head chunks: [P, KG, 2, C]
    y0p = np.ascontiguousarray(yq[:, :, :, 0:C0].transpose(1, 0, 2, 3))
    y1p = np.ascontiguousarray(yq[:, :, :, C0 : C0 + C1].transpose(1, 0, 2, 3))

    in_maps = []
    for c in range(NCORES):
        Xs = X[c * RPC : (c + 1) * RPC]
        xt_km = np.empty((D, RPC), dtype=NP_FP8)
        xt_km[:DF] = Xs.T[:DF].astype(NP_FP8)
        xt_km[DF:] = np.float32(1.0)
        xq = _dr_km(xt_km)                   # [KG, P, 2, RPC]
        # m-major pack: [P, MT, KG, 2, 128]
        xtm = np.ascontiguousarray(
            xq.reshape(KG, P, 2, MT, P).transpose(1, 3, 0, 2, 4)
        )
        sqx_c = ((Xs.astype(np.float64) ** 2).sum(axis=1) - float(D)).astype(
            np.float32
        )
        sqx_pm = np.ascontiguousarray(sqx_c.reshape(MT, P).T)
        m = {f"yt{kg}": np.ascontiguousarray(yq[kg]) for kg in range(KG)}
        m.update({"xtm": xtm, "y0": y0p, "y1": y1p, "sqx": sqx_pm})
        in_maps.append(m)
    return in_maps


def kernel(input, target):
    X = np.ascontiguousarray(np.asarray(input, dtype=np.float32))
    Y = np.ascontiguousarray(np.asarray(target, dtype=np.float32))
    assert X.shape == (B, D) and Y.shape == (B, D)

    nc = _get_nc()
    in_maps = _prep_inputs(X, Y)
    try:
        res = run_bass_kernel_spmd(nc, in_maps, core_ids=list(range(NCORES))).results
    except Exception:
        # a prior process can leave a core wedged; one retry clears it
        res = run_bass_kernel_spmd(nc, in_maps, core_ids=list(range(NCORES))).results

    off = np.float64(2.0 * D)
    row_sum = np.float64(0.0)
    col_parts = []
    for r in res:
        # fold the raw last-chunk tiles into both partial-min outputs
        raw = r["mseL"].reshape(P, MT, CHUNK).astype(np.float32)
        rp = r["rowpart"].reshape(P, MT, CHUNK).astype(np.float32).min(axis=2)
        rm = np.minimum(rp, raw.min(axis=2))
        row_sum += (rm.astype(np.float64) + off).sum()
        cm = r["colmin"].astype(np.float32)[:, : B - CHUNK]
        cm7 = raw.min(axis=1)  # [P, CHUNK]: col partial over this core's m
        col_parts.append(np.concatenate([cm.min(axis=0), cm7.min(axis=0)]))
    col_min = np.min(np.stack(col_parts), axis=0).astype(np.float64) + off
    loss = (row_sum + col_min.sum()) / D / (2 * B)
    return np.asarray(loss, dtype=np.float32)
